# revision 10
# baseline (speedup 1.0000x reference)
# Trainium2 Bass kernel for nn_CBA — hybrid-precision, software-pipelined.
#
# Same algorithm as kernel.py (scalar-gather restructure; see there), but
# the two score streams use different engines so no engine exceeds the
# ~358 GB/s DMA roofline (~134 us for the 48 MB/core of 16-bit streams):
#   - lba half ships f16:  DVE tensor_tensor mult (2x mode, 594 ns/tile)
#     + ScalarE activation-accumulate (1x, ~1.0 us/tile).
#   - emb half ships int16: fused scalar_tensor_tensor mult+accum on DVE
#     (1x, ~1.1 us/tile) — int16 keeps the score noise low.
#   DVE ~110 us, ACT ~75 us, both under the DMA roof.
# Phases are software-pipelined per batch (B finalized one batch late,
# outputs two late) so the rnn stream interleaves with the next batch's
# score stream and no engine queue blocks another.
#
# Numpy-simulated rel err vs fp32 reference: 7.5e-3 (tolerance 2e-2).

import numpy as np
from contextlib import ExitStack

B, L, E, R = 32, 2048, 1024, 1024
NCORES = 8
BPC = B // NCORES          # batches per core
F = E + R                  # concat feature dim
EPS = 1e-7
NLT = L // 128             # l-tiles per batch (16)
CHA = 8                    # l-tiles per x-stream DMA chunk (4 MB)
CHR = 8                    # l-tiles per fp16 rnn-stream DMA chunk (2 MB)
QX = 6.0 / 32767.0         # int16 quant step for embs/W streams
WCLIP = 136.0              # |wsum| range covered by the int16 wrep
SCALE_TTR = QX * WCLIP / 32767.0   # int16*int16 product -> true units
SCALE_WQ = QX * 32767.0 / WCLIP    # raw fp32 wsum accum -> int16 wrep

_PROG = None
LAST_RESULTS = None


def _build(rep=1, timing=False):
    import concourse.mybir as mybir
    import concourse.tile as tile
    from concourse import bacc, bass_isa
    from concourse.masks import make_identity

    f32 = mybir.dt.float32
    f16 = mybir.dt.float16
    i16 = mybir.dt.int16
    u16 = mybir.dt.uint16
    AOP = mybir.AluOpType
    AF = mybir.ActivationFunctionType

    nc = bacc.Bacc("TRN2", debug=False, enable_asserts=False,
                   target_bir_lowering=False, num_devices=NCORES)

    big = "Internal" if timing else "ExternalInput"
    xq = nc.dram_tensor("xq", [BPC, L, F], u16, kind=big).ap()
    rnn = nc.dram_tensor("rnn", [BPC, L, R], f16, kind=big).ap()
    wT = nc.dram_tensor("wT", [R, F], i16, kind=big).ap()
    idxs = nc.dram_tensor("idxs", [BPC, 128, NLT], u16, kind="ExternalInput").ap()
    out = nc.dram_tensor("out", [BPC, R], f32, kind="ExternalOutput").ap()

    with tile.TileContext(nc) as tc, ExitStack() as ctx:
        cpool = ctx.enter_context(tc.tile_pool(name="const", bufs=1))
        identity = cpool.tile([128, 128], f32)
        make_identity(nc, identity)
        ones = cpool.tile([128, 1], f16)
        nc.vector.memset(ones, 1.0)
        # wsum[f] = sum_r W[f, r]; lba half kept f16 in true units,
        # emb half requantized to int16 (true value times 32767/WCLIP).
        wrepf = cpool.tile([128, R], f16)
        wrepi = cpool.tile([128, E], i16)
        with tc.tile_pool(name="wstage", bufs=1) as wpool:
            hr = R // 256
            waccs = []
            for hh in range(2):
                wst = wpool.tile([128, hr, F], i16, tag=f"wst{hh}")
                nc.sync.dma_start(
                    wst, wT[hh * (R // 2):(hh + 1) * (R // 2), :]
                    .rearrange("(a p) f -> p a f", p=128))
                wacc = wpool.tile([128, F], f32, tag=f"wacc{hh}")
                nc.vector.tensor_reduce(wacc, wst.rearrange("p a f -> p f a"),
                                        axis=mybir.AxisListType.X, op=AOP.add)
                waccs.append(wacc)
            wboth = wpool.tile([128, F], f32)
            nc.vector.tensor_add(wboth, waccs[0], waccs[1])
            wsum = wpool.tile([128, F], f32)
            nc.gpsimd.partition_all_reduce(wsum, wboth, channels=128,
                                           reduce_op=bass_isa.ReduceOp.add)
            nc.scalar.activation(wrepf, wsum[:, 0:R], AF.Copy, scale=QX)
            nc.scalar.activation(wrepi, wsum[:, R:F], AF.Copy, scale=SCALE_WQ)

        spool = ctx.enter_context(tc.tile_pool(name="streams", bufs=4))
        scratch = ctx.enter_context(tc.tile_pool(name="scratch", bufs=2))
        tabs = ctx.enter_context(tc.tile_pool(name="tabs", bufs=2))
        small = ctx.enter_context(tc.tile_pool(name="small", bufs=2))
        opool = ctx.enter_context(tc.tile_pool(name="outp", bufs=2))
        psmm = ctx.enter_context(tc.tile_pool(name="psmm", bufs=4, space="PSUM"))
        psden = ctx.enter_context(tc.tile_pool(name="psden", bufs=2, space="PSUM"))
        pstp = ctx.enter_context(tc.tile_pool(name="pstp", bufs=2, space="PSUM"))

        for _ in range(rep):
            s_lbas, s_embs, ws = [None] * BPC, [None] * BPC, [None] * BPC
            psAs, psBs, psDs = [None] * BPC, [None] * BPC, [None] * BPC

            def a_chunk(b, c):
                rows = slice(c * CHA * 128, (c + 1) * CHA * 128)
                xt = spool.tile([128, CHA, F], u16, tag="x", bufs=2)
                nc.sync.dma_start(
                    xt, xq[b, rows, :].rearrange("(a p) f -> p a f", p=128))
                for a in range(CHA):
                    t = c * CHA + a
                    # lba half, f16: DVE 2x mult then ScalarE accumulate
                    pf = scratch.tile([128, R], f16, tag="prodf", bufs=8)
                    nc.vector.tensor_mul(pf, xt[:, a, 0:R].bitcast(f16), wrepf)
                    dump = scratch.tile([128, R], f16, tag="dump", bufs=2)
                    nc.scalar.activation(dump, pf, AF.Copy,
                                         accum_out=s_lbas[b][:, t:t + 1])
                    # emb half, int16: fused mult+accumulate on DVE
                    pi = scratch.tile([128, E], f16, tag="prodi")
                    nc.vector.scalar_tensor_tensor(
                        pi, xt[:, a, R:F].bitcast(i16), SCALE_TTR, wrepi,
                        op0=AOP.mult, op1=AOP.mult,
                        accum_out=s_embs[b][:, t:t + 1])

            def b_front(b):
                flat = tabs.tile([1, L], f32, tag="flat")
                nc.scalar.dma_start(flat.rearrange("o (p t) -> o p t", p=128),
                                    s_lbas[b])
                table = tabs.tile([128, L], f32, tag="table")
                nc.gpsimd.partition_broadcast(table, flat, channels=128)
                idxt = small.tile([128, NLT], u16, tag="idx")
                nc.scalar.dma_start(idxt, idxs[b])
                G = small.tile([128, 256], f32, tag="G")
                nc.gpsimd.indirect_copy(G, table, idxt, True)
                T0 = pstp.tile([128, 128], f32, tag="tp")
                nc.tensor.transpose(T0, G[:, 0:128], identity)
                T1 = pstp.tile([128, 128], f32, tag="tp")
                nc.tensor.transpose(T1, G[:, 128:256], identity)
                return T0, T1

            def b_fin(b, T0, T1):
                scl = small.tile([128, NLT], f32, tag="scl")
                scl3 = scl.rearrange("p (a two) -> p a two", two=2)
                nc.vector.tensor_copy(
                    scl3[:, :, 0:1],
                    T0.rearrange("p (a j) -> p a j", j=16)[:, :, 0:1])
                nc.vector.tensor_copy(
                    scl3[:, :, 1:2],
                    T1.rearrange("p (a j) -> p a j", j=16)[:, :, 0:1])
                scores = small.tile([128, NLT], f32, tag="scores")
                nc.vector.tensor_add(scores, scl, s_embs[b])
                th = small.tile([128, NLT], f32, tag="th")
                nc.scalar.activation(th, scores, AF.Tanh)
                w = small.tile([128, NLT], f16, tag=f"w{b}")
                nc.scalar.activation(w, th, AF.Exp)
                ws[b] = w

            rts = [None] * BPC

            def c_rnn(b):
                tiles = []
                for c in range(NLT // CHR):
                    rows = slice(c * CHR * 128, (c + 1) * CHR * 128)
                    rt = spool.tile([128, CHR, R], f16, tag="rnn", bufs=4)
                    nc.gpsimd.dma_start(
                        rt, rnn[b, rows, :].rearrange("(a p) f -> p a f", p=128))
                    tiles.append(rt)
                rts[b] = tiles

            def c_mm(b):
                w = ws[b]
                psA = psmm.tile([1, 512], f32, tag="mm")
                psB = psmm.tile([1, 512], f32, tag="mm")
                psD = psden.tile([1, 1], f32, tag="den")
                psAs[b], psBs[b], psDs[b] = psA, psB, psD
                for c in range(NLT // CHR):
                    rt = rts[b][c]
                    for a in range(CHR):
                        t = c * CHR + a
                        st, sp = (t == 0), (t == NLT - 1)
                        wcol = w[:, t:t + 1]
                        nc.tensor.matmul(psA, wcol, rt[:, a, 0:512], start=st, stop=sp)
                        nc.tensor.matmul(psB, wcol, rt[:, a, 512:1024], start=st, stop=sp)
                        nc.tensor.matmul(psD, wcol, ones, start=st, stop=sp)

            def c_out(b):
                den = small.tile([1, 1], f32, tag="den_sb")
                nc.vector.tensor_scalar_add(den, psDs[b], EPS)
                rinv = small.tile([1, 1], f32, tag="rinv")
                nc.vector.reciprocal(rinv, den)
                ot = opool.tile([1, R], f32, tag="ot")
                nc.scalar.activation(ot[:, 0:512], psAs[b], AF.Copy, scale=rinv)
                nc.scalar.activation(ot[:, 512:1024], psBs[b], AF.Copy, scale=rinv)
                nc.scalar.dma_start(out[b:b + 1, :], ot)

            # software pipeline: A(b) || B_fin(b-1)+C_mm(b-1) || C_out(b-2)
            tps = [None] * BPC
            for b in range(BPC):
                s_lbas[b] = small.tile([128, NLT], f32, tag=f"slba{b}",
                                       name=f"slba{b}")
                s_embs[b] = small.tile([128, NLT], f32, tag=f"semb{b}",
                                       name=f"semb{b}")
                a_chunk(b, 0)
                c_rnn(b)
                if b >= 1:
                    b_fin(b - 1, *tps[b - 1])
                    c_mm(b - 1)
                if b >= 2:
                    c_out(b - 2)
                for c in range(1, NLT // CHA):
                    a_chunk(b, c)
                tps[b] = b_front(b)
            b_fin(BPC - 1, *tps[BPC - 1])
            c_mm(BPC - 1)
            c_out(BPC - 2)
            c_out(BPC - 1)

    nc.compile()
    return nc


def _get_prog():
    global _PROG
    if _PROG is None:
        _PROG = _build()
    return _PROG


def _qi16(x, q):
    return np.clip(np.round(x * (1.0 / q)), -32767, 32767).astype(np.int16)


def _marshal(embs, prnt_indices, lba_out, rnn_out, W):
    """Host-side input layout: shard over batch, lba->f16 / embs->int16
    into one uint16 container, rnn->fp16, W->int16 transposed, remap idx."""
    lba_q = np.asarray(lba_out, dtype=np.float32).astype(np.float16).view(np.uint16)
    emb_q = _qi16(np.asarray(embs, dtype=np.float32), QX).view(np.uint16)
    xq = np.ascontiguousarray(np.concatenate([lba_q, emb_q], axis=-1))
    rnn = np.asarray(rnn_out, dtype=np.float32).astype(np.float16)
    wTq = np.ascontiguousarray(_qi16(np.asarray(W, dtype=np.float32), QX).T)
    idx = np.asarray(prnt_indices).astype(np.int64)

    pos = ((idx % 128) * NLT + idx // 128).astype(np.uint16)  # [B, L]
    A = pos.reshape(B, 8, 16, 16)
    idxs_w = np.ascontiguousarray(A.transpose(0, 1, 3, 2).reshape(B, 128, NLT))

    in_maps = []
    for c in range(NCORES):
        s = slice(c * BPC, (c + 1) * BPC)
        in_maps.append({
            "xq": xq[s],
            "rnn": rnn[s],
            "wT": wTq,
            "idxs": idxs_w[s],
        })
    return in_maps


def kernel(embs, prnt_indices, lba_out, rnn_out, W):
    global LAST_RESULTS
    from concourse.bass_utils import run_bass_kernel_spmd

    nc = _get_prog()
    in_maps = _marshal(embs, prnt_indices, lba_out, rnn_out, W)
    res = run_bass_kernel_spmd(nc, in_maps, core_ids=list(range(NCORES)))
    LAST_RESULTS = res
    out = np.concatenate([r["out"] for r in res.results], axis=0)
    return out.astype(np.float32)


# revision 11
# speedup vs baseline: 1.0316x; 1.0316x over previous
# Trainium2 Bass kernel for nn_CBA — hybrid-precision, software-pipelined.
#
# Same algorithm as kernel.py (scalar-gather restructure; see there), but
# the two score streams use different engines so no engine exceeds the
# ~358 GB/s DMA roofline (~134 us for the 48 MB/core of 16-bit streams):
#   - lba half ships f16:  DVE tensor_tensor mult (2x mode, 594 ns/tile)
#     + ScalarE activation-accumulate (1x, ~1.0 us/tile).
#   - emb half ships int16: fused scalar_tensor_tensor mult+accum on DVE
#     (1x, ~1.1 us/tile) — int16 keeps the score noise low.
#   DVE ~110 us, ACT ~75 us, both under the DMA roof.
# Phases are software-pipelined per batch (B finalized one batch late,
# outputs two late) so the rnn stream interleaves with the next batch's
# score stream and no engine queue blocks another.
#
# Numpy-simulated rel err vs fp32 reference: 7.5e-3 (tolerance 2e-2).

import numpy as np
from contextlib import ExitStack

B, L, E, R = 32, 2048, 1024, 1024
NCORES = 8
BPC = B // NCORES          # batches per core
F = E + R                  # concat feature dim
EPS = 1e-7
NLT = L // 128             # l-tiles per batch (16)
CHA = 8                    # l-tiles per x-stream DMA chunk (4 MB)
CHR = 8                    # l-tiles per fp16 rnn-stream DMA chunk (2 MB)
QX = 6.0 / 32767.0         # int16 quant step for embs/W streams
WCLIP = 136.0              # |wsum| range covered by the int16 wrep
SCALE_TTR = QX * WCLIP / 32767.0   # int16*int16 product -> true units
SCALE_WQ = QX * 32767.0 / WCLIP    # raw fp32 wsum accum -> int16 wrep

_PROG = None
LAST_RESULTS = None


def _build(rep=1, timing=False):
    import concourse.mybir as mybir
    import concourse.tile as tile
    from concourse import bacc, bass_isa
    from concourse.masks import make_identity

    f32 = mybir.dt.float32
    f16 = mybir.dt.float16
    i16 = mybir.dt.int16
    u16 = mybir.dt.uint16
    AOP = mybir.AluOpType
    AF = mybir.ActivationFunctionType

    nc = bacc.Bacc("TRN2", debug=False, enable_asserts=False,
                   target_bir_lowering=False, num_devices=NCORES)

    big = "Internal" if timing else "ExternalInput"
    xq = nc.dram_tensor("xq", [BPC, L, F], u16, kind=big).ap()
    rnn = nc.dram_tensor("rnn", [BPC, L, R], f16, kind=big).ap()
    wT = nc.dram_tensor("wT", [R, F], i16, kind=big).ap()
    idxs = nc.dram_tensor("idxs", [BPC, 128, NLT], u16, kind="ExternalInput").ap()
    out = nc.dram_tensor("out", [BPC, R], f32, kind="ExternalOutput").ap()

    with tile.TileContext(nc) as tc, ExitStack() as ctx:
        cpool = ctx.enter_context(tc.tile_pool(name="const", bufs=1))
        identity = cpool.tile([128, 128], f32)
        make_identity(nc, identity)
        ones = cpool.tile([128, 1], f16)
        nc.vector.memset(ones, 1.0)
        # wsum[f] = sum_r W[f, r]; lba half kept f16 in true units,
        # emb half requantized to int16 (true value times 32767/WCLIP).
        wrepf = cpool.tile([128, R], f16)
        wrepi = cpool.tile([128, E], i16)
        with tc.tile_pool(name="wstage", bufs=1) as wpool:
            hr = R // 256
            waccs = []
            for hh in range(2):
                wst = wpool.tile([128, hr, F], i16, tag=f"wst{hh}")
                nc.sync.dma_start(
                    wst, wT[hh * (R // 2):(hh + 1) * (R // 2), :]
                    .rearrange("(a p) f -> p a f", p=128))
                wacc = wpool.tile([128, F], f32, tag=f"wacc{hh}")
                nc.vector.tensor_reduce(wacc, wst.rearrange("p a f -> p f a"),
                                        axis=mybir.AxisListType.X, op=AOP.add)
                waccs.append(wacc)
            wboth = wpool.tile([128, F], f32)
            nc.vector.tensor_add(wboth, waccs[0], waccs[1])
            wsum = wpool.tile([128, F], f32)
            nc.gpsimd.partition_all_reduce(wsum, wboth, channels=128,
                                           reduce_op=bass_isa.ReduceOp.add)
            nc.scalar.activation(wrepf, wsum[:, 0:R], AF.Copy, scale=QX)
            nc.scalar.activation(wrepi, wsum[:, R:F], AF.Copy, scale=SCALE_WQ)

        spool = ctx.enter_context(tc.tile_pool(name="streams", bufs=4))
        scratch = ctx.enter_context(tc.tile_pool(name="scratch", bufs=2))
        tabs = ctx.enter_context(tc.tile_pool(name="tabs", bufs=1))
        small = ctx.enter_context(tc.tile_pool(name="small", bufs=2))
        opool = ctx.enter_context(tc.tile_pool(name="outp", bufs=2))
        psmm = ctx.enter_context(tc.tile_pool(name="psmm", bufs=4, space="PSUM"))
        psden = ctx.enter_context(tc.tile_pool(name="psden", bufs=2, space="PSUM"))
        pstp = ctx.enter_context(tc.tile_pool(name="pstp", bufs=2, space="PSUM"))

        for _ in range(rep):
            s_lbas, s_embs, ws = [None] * BPC, [None] * BPC, [None] * BPC
            psAs, psBs, psDs = [None] * BPC, [None] * BPC, [None] * BPC

            def a_chunk(b, c):
                rows = slice(c * CHA * 128, (c + 1) * CHA * 128)
                xt = spool.tile([128, CHA, F], u16, tag="x", bufs=3)
                nc.sync.dma_start(
                    xt, xq[b, rows, :].rearrange("(a p) f -> p a f", p=128))
                for a in range(CHA):
                    t = c * CHA + a
                    # lba half, f16: DVE 2x mult then ScalarE accumulate
                    pf = scratch.tile([128, R], f16, tag="prodf", bufs=8)
                    nc.vector.tensor_mul(pf, xt[:, a, 0:R].bitcast(f16), wrepf)
                    dump = scratch.tile([128, R], f16, tag="dump", bufs=2)
                    nc.scalar.activation(dump, pf, AF.Copy,
                                         accum_out=s_lbas[b][:, t:t + 1])
                    # emb half, int16: fused mult+accumulate on DVE
                    pi = scratch.tile([128, E], f16, tag="prodi")
                    nc.vector.scalar_tensor_tensor(
                        pi, xt[:, a, R:F].bitcast(i16), SCALE_TTR, wrepi,
                        op0=AOP.mult, op1=AOP.mult,
                        accum_out=s_embs[b][:, t:t + 1])

            def b_front(b):
                flat = tabs.tile([1, L], f32, tag="flat")
                nc.scalar.dma_start(flat.rearrange("o (p t) -> o p t", p=128),
                                    s_lbas[b])
                table = tabs.tile([128, L], f32, tag="table")
                nc.gpsimd.partition_broadcast(table, flat, channels=128)
                idxt = small.tile([128, NLT], u16, tag="idx")
                nc.scalar.dma_start(idxt, idxs[b])
                G = small.tile([128, 256], f32, tag="G")
                nc.gpsimd.indirect_copy(G, table, idxt, True)
                T0 = pstp.tile([128, 128], f32, tag="tp")
                nc.tensor.transpose(T0, G[:, 0:128], identity)
                T1 = pstp.tile([128, 128], f32, tag="tp")
                nc.tensor.transpose(T1, G[:, 128:256], identity)
                return T0, T1

            def b_fin(b, T0, T1):
                scl = small.tile([128, NLT], f32, tag="scl")
                scl3 = scl.rearrange("p (a two) -> p a two", two=2)
                nc.vector.tensor_copy(
                    scl3[:, :, 0:1],
                    T0.rearrange("p (a j) -> p a j", j=16)[:, :, 0:1])
                nc.vector.tensor_copy(
                    scl3[:, :, 1:2],
                    T1.rearrange("p (a j) -> p a j", j=16)[:, :, 0:1])
                scores = small.tile([128, NLT], f32, tag="scores")
                nc.vector.tensor_add(scores, scl, s_embs[b])
                th = small.tile([128, NLT], f32, tag="th")
                nc.scalar.activation(th, scores, AF.Tanh)
                w = small.tile([128, NLT], f16, tag=f"w{b}")
                nc.scalar.activation(w, th, AF.Exp)
                ws[b] = w

            rts = [None] * BPC

            def c_rnn(b):
                tiles = []
                for c in range(NLT // CHR):
                    rows = slice(c * CHR * 128, (c + 1) * CHR * 128)
                    rt = spool.tile([128, CHR, R], f16, tag="rnn", bufs=2)
                    nc.gpsimd.dma_start(
                        rt, rnn[b, rows, :].rearrange("(a p) f -> p a f", p=128))
                    tiles.append(rt)
                rts[b] = tiles

            def c_mm(b):
                w = ws[b]
                psA = psmm.tile([1, 512], f32, tag="mm")
                psB = psmm.tile([1, 512], f32, tag="mm")
                psD = psden.tile([1, 1], f32, tag="den")
                psAs[b], psBs[b], psDs[b] = psA, psB, psD
                for c in range(NLT // CHR):
                    rt = rts[b][c]
                    for a in range(CHR):
                        t = c * CHR + a
                        st, sp = (t == 0), (t == NLT - 1)
                        wcol = w[:, t:t + 1]
                        nc.tensor.matmul(psA, wcol, rt[:, a, 0:512], start=st, stop=sp)
                        nc.tensor.matmul(psB, wcol, rt[:, a, 512:1024], start=st, stop=sp)
                        nc.tensor.matmul(psD, wcol, ones, start=st, stop=sp)

            def c_out(b):
                den = small.tile([1, 1], f32, tag="den_sb")
                nc.vector.tensor_scalar_add(den, psDs[b], EPS)
                rinv = small.tile([1, 1], f32, tag="rinv")
                nc.vector.reciprocal(rinv, den)
                ot = opool.tile([1, R], f32, tag="ot")
                nc.scalar.activation(ot[:, 0:512], psAs[b], AF.Copy, scale=rinv)
                nc.scalar.activation(ot[:, 512:1024], psBs[b], AF.Copy, scale=rinv)
                nc.scalar.dma_start(out[b:b + 1, :], ot)

            # software pipeline: A(b) || B_fin(b-1)+C_mm(b-1) || C_out(b-2)
            tps = [None] * BPC
            for b in range(BPC):
                s_lbas[b] = small.tile([128, NLT], f32, tag=f"slba{b}",
                                       name=f"slba{b}")
                s_embs[b] = small.tile([128, NLT], f32, tag=f"semb{b}",
                                       name=f"semb{b}")
                a_chunk(b, 0)
                c_rnn(b)
                if b >= 1:
                    b_fin(b - 1, *tps[b - 1])
                    c_mm(b - 1)
                if b >= 2:
                    c_out(b - 2)
                for c in range(1, NLT // CHA):
                    a_chunk(b, c)
                tps[b] = b_front(b)
            b_fin(BPC - 1, *tps[BPC - 1])
            c_mm(BPC - 1)
            c_out(BPC - 2)
            c_out(BPC - 1)

    nc.compile()
    return nc


def _get_prog():
    global _PROG
    if _PROG is None:
        _PROG = _build()
    return _PROG


def _qi16(x, q):
    return np.clip(np.round(x * (1.0 / q)), -32767, 32767).astype(np.int16)


def _marshal(embs, prnt_indices, lba_out, rnn_out, W):
    """Host-side input layout: shard over batch, lba->f16 / embs->int16
    into one uint16 container, rnn->fp16, W->int16 transposed, remap idx."""
    lba_q = np.asarray(lba_out, dtype=np.float32).astype(np.float16).view(np.uint16)
    emb_q = _qi16(np.asarray(embs, dtype=np.float32), QX).view(np.uint16)
    xq = np.ascontiguousarray(np.concatenate([lba_q, emb_q], axis=-1))
    rnn = np.asarray(rnn_out, dtype=np.float32).astype(np.float16)
    wTq = np.ascontiguousarray(_qi16(np.asarray(W, dtype=np.float32), QX).T)
    idx = np.asarray(prnt_indices).astype(np.int64)

    pos = ((idx % 128) * NLT + idx // 128).astype(np.uint16)  # [B, L]
    A = pos.reshape(B, 8, 16, 16)
    idxs_w = np.ascontiguousarray(A.transpose(0, 1, 3, 2).reshape(B, 128, NLT))

    in_maps = []
    for c in range(NCORES):
        s = slice(c * BPC, (c + 1) * BPC)
        in_maps.append({
            "xq": xq[s],
            "rnn": rnn[s],
            "wT": wTq,
            "idxs": idxs_w[s],
        })
    return in_maps


def kernel(embs, prnt_indices, lba_out, rnn_out, W):
    global LAST_RESULTS
    from concourse.bass_utils import run_bass_kernel_spmd

    nc = _get_prog()
    in_maps = _marshal(embs, prnt_indices, lba_out, rnn_out, W)
    res = run_bass_kernel_spmd(nc, in_maps, core_ids=list(range(NCORES)))
    LAST_RESULTS = res
    out = np.concatenate([r["out"] for r in res.results], axis=0)
    return out.astype(np.float32)


# revision 14
# speedup vs baseline: 1.0588x; 1.0264x over previous
# Trainium2 Bass kernel for nn_CBA — hybrid-precision, software-pipelined.
#
# Same algorithm as kernel.py (scalar-gather restructure; see there), but
# the two score streams use different engines so no engine exceeds the
# ~358 GB/s DMA roofline (~134 us for the 48 MB/core of 16-bit streams):
#   - lba half ships f16:  DVE tensor_tensor mult (2x mode, 594 ns/tile)
#     + ScalarE activation-accumulate (1x, ~1.0 us/tile).
#   - emb half ships int16: fused scalar_tensor_tensor mult+accum on DVE
#     (1x, ~1.1 us/tile) — int16 keeps the score noise low.
#   DVE ~110 us, ACT ~75 us, both under the DMA roof.
# Phases are software-pipelined per batch (B finalized one batch late,
# outputs two late) so the rnn stream interleaves with the next batch's
# score stream and no engine queue blocks another.
#
# Numpy-simulated rel err vs fp32 reference: 7.5e-3 (tolerance 2e-2).

import numpy as np
from contextlib import ExitStack

B, L, E, R = 32, 2048, 1024, 1024
NCORES = 8
BPC = B // NCORES          # batches per core
F = E + R                  # concat feature dim
EPS = 1e-7
NLT = L // 128             # l-tiles per batch (16)
CHA = 4                    # l-tiles per x-stream DMA chunk (2 MB)
CHR = 8                    # l-tiles per fp16 rnn-stream DMA chunk (2 MB)
QX = 6.0 / 32767.0         # int16 quant step for embs/W streams
WCLIP = 136.0              # |wsum| range covered by the int16 wrep
SCALE_TTR = QX * WCLIP / 32767.0   # int16*int16 product -> true units
SCALE_WQ = QX * 32767.0 / WCLIP    # raw fp32 wsum accum -> int16 wrep

_PROG = None
LAST_RESULTS = None


def _build(rep=1, timing=False):
    import concourse.mybir as mybir
    import concourse.tile as tile
    from concourse import bacc, bass_isa
    from concourse.masks import make_identity

    f32 = mybir.dt.float32
    f16 = mybir.dt.float16
    i16 = mybir.dt.int16
    u16 = mybir.dt.uint16
    AOP = mybir.AluOpType
    AF = mybir.ActivationFunctionType

    nc = bacc.Bacc("TRN2", debug=False, enable_asserts=False,
                   target_bir_lowering=False, num_devices=NCORES)

    big = "Internal" if timing else "ExternalInput"
    xq = nc.dram_tensor("xq", [BPC, L, F], u16, kind=big).ap()
    rnn = nc.dram_tensor("rnn", [BPC, L, R], f16, kind=big).ap()
    wT = nc.dram_tensor("wT", [R, F], i16, kind=big).ap()
    idxs = nc.dram_tensor("idxs", [BPC, 128, NLT], u16, kind="ExternalInput").ap()
    out = nc.dram_tensor("out", [BPC, R], f32, kind="ExternalOutput").ap()

    with tile.TileContext(nc) as tc, ExitStack() as ctx:
        cpool = ctx.enter_context(tc.tile_pool(name="const", bufs=1))
        identity = cpool.tile([128, 128], f32)
        make_identity(nc, identity)
        ones = cpool.tile([128, 1], f16)
        nc.vector.memset(ones, 1.0)
        # wsum[f] = sum_r W[f, r]; lba half kept f16 in true units,
        # emb half requantized to int16 (true value times 32767/WCLIP).
        wrepf = cpool.tile([128, R], f16)
        wrepi = cpool.tile([128, E], i16)
        with tc.tile_pool(name="wstage", bufs=1) as wpool:
            hr = R // 256
            waccs = []
            for hh in range(2):
                wst = wpool.tile([128, hr, F], i16, tag=f"wst{hh}")
                nc.sync.dma_start(
                    wst, wT[hh * (R // 2):(hh + 1) * (R // 2), :]
                    .rearrange("(a p) f -> p a f", p=128))
                wacc = wpool.tile([128, F], f32, tag=f"wacc{hh}")
                nc.vector.tensor_reduce(wacc, wst.rearrange("p a f -> p f a"),
                                        axis=mybir.AxisListType.X, op=AOP.add)
                waccs.append(wacc)
            wboth = wpool.tile([128, F], f32)
            nc.vector.tensor_add(wboth, waccs[0], waccs[1])
            wsum = wpool.tile([128, F], f32)
            nc.gpsimd.partition_all_reduce(wsum, wboth, channels=128,
                                           reduce_op=bass_isa.ReduceOp.add)
            nc.scalar.activation(wrepf, wsum[:, 0:R], AF.Copy, scale=QX)
            nc.scalar.activation(wrepi, wsum[:, R:F], AF.Copy, scale=SCALE_WQ)

        spool = ctx.enter_context(tc.tile_pool(name="streams", bufs=4))
        scratch = ctx.enter_context(tc.tile_pool(name="scratch", bufs=2))
        tabs = ctx.enter_context(tc.tile_pool(name="tabs", bufs=1))
        small = ctx.enter_context(tc.tile_pool(name="small", bufs=2))
        opool = ctx.enter_context(tc.tile_pool(name="outp", bufs=2))
        psmm = ctx.enter_context(tc.tile_pool(name="psmm", bufs=4, space="PSUM"))
        psden = ctx.enter_context(tc.tile_pool(name="psden", bufs=2, space="PSUM"))
        pstp = ctx.enter_context(tc.tile_pool(name="pstp", bufs=2, space="PSUM"))

        for _ in range(rep):
            s_lbas, s_embs, ws = [None] * BPC, [None] * BPC, [None] * BPC
            psAs, psBs, psDs = [None] * BPC, [None] * BPC, [None] * BPC

            def a_chunk(b, c):
                rows = slice(c * CHA * 128, (c + 1) * CHA * 128)
                xt = spool.tile([128, CHA, F], u16, tag="x", bufs=6)
                nc.sync.dma_start(
                    xt, xq[b, rows, :].rearrange("(a p) f -> p a f", p=128))
                for a in range(CHA):
                    t = c * CHA + a
                    # lba half, f16: DVE 2x mult then ScalarE accumulate
                    pf = scratch.tile([128, R], f16, tag="prodf", bufs=10)
                    nc.vector.tensor_mul(pf, xt[:, a, 0:R].bitcast(f16), wrepf)
                    dump = scratch.tile([128, R], f16, tag="dump", bufs=2)
                    nc.scalar.activation(dump, pf, AF.Copy,
                                         accum_out=s_lbas[b][:, t:t + 1])
                    # emb half, int16: fused mult+accumulate on DVE
                    pi = scratch.tile([128, E], f16, tag="prodi")
                    nc.vector.scalar_tensor_tensor(
                        pi, xt[:, a, R:F].bitcast(i16), SCALE_TTR, wrepi,
                        op0=AOP.mult, op1=AOP.mult,
                        accum_out=s_embs[b][:, t:t + 1])

            def b_front(b):
                flat = tabs.tile([1, L], f32, tag="flat")
                nc.scalar.dma_start(flat.rearrange("o (p t) -> o p t", p=128),
                                    s_lbas[b])
                table = tabs.tile([128, L], f32, tag="table")
                nc.gpsimd.partition_broadcast(table, flat, channels=128)
                idxt = small.tile([128, NLT], u16, tag="idx")
                nc.scalar.dma_start(idxt, idxs[b])
                G = small.tile([128, 256], f32, tag="G")
                nc.gpsimd.indirect_copy(G, table, idxt, True)
                T0 = pstp.tile([128, 128], f32, tag="tp")
                nc.tensor.transpose(T0, G[:, 0:128], identity)
                T1 = pstp.tile([128, 128], f32, tag="tp")
                nc.tensor.transpose(T1, G[:, 128:256], identity)
                return T0, T1

            def b_fin(b, T0, T1):
                scl = small.tile([128, NLT], f32, tag="scl")
                scl3 = scl.rearrange("p (a two) -> p a two", two=2)
                nc.vector.tensor_copy(
                    scl3[:, :, 0:1],
                    T0.rearrange("p (a j) -> p a j", j=16)[:, :, 0:1])
                nc.vector.tensor_copy(
                    scl3[:, :, 1:2],
                    T1.rearrange("p (a j) -> p a j", j=16)[:, :, 0:1])
                scores = small.tile([128, NLT], f32, tag="scores")
                nc.vector.tensor_add(scores, scl, s_embs[b])
                th = small.tile([128, NLT], f32, tag="th")
                nc.scalar.activation(th, scores, AF.Tanh)
                w = small.tile([128, NLT], f16, tag=f"w{b}")
                nc.scalar.activation(w, th, AF.Exp)
                ws[b] = w

            rts = [None] * BPC

            def c_rnn(b):
                tiles = []
                for c in range(NLT // CHR):
                    rows = slice(c * CHR * 128, (c + 1) * CHR * 128)
                    rt = spool.tile([128, CHR, R], f16, tag="rnn", bufs=2)
                    nc.gpsimd.dma_start(
                        rt, rnn[b, rows, :].rearrange("(a p) f -> p a f", p=128))
                    tiles.append(rt)
                rts[b] = tiles

            def c_mm(b):
                w = ws[b]
                psA = psmm.tile([1, 512], f32, tag="mm")
                psB = psmm.tile([1, 512], f32, tag="mm")
                psD = psden.tile([1, 1], f32, tag="den")
                psAs[b], psBs[b], psDs[b] = psA, psB, psD
                for c in range(NLT // CHR):
                    rt = rts[b][c]
                    for a in range(CHR):
                        t = c * CHR + a
                        st, sp = (t == 0), (t == NLT - 1)
                        wcol = w[:, t:t + 1]
                        nc.tensor.matmul(psA, wcol, rt[:, a, 0:512], start=st, stop=sp)
                        nc.tensor.matmul(psB, wcol, rt[:, a, 512:1024], start=st, stop=sp)
                        nc.tensor.matmul(psD, wcol, ones, start=st, stop=sp)

            def c_out(b):
                den = small.tile([1, 1], f32, tag="den_sb")
                nc.vector.tensor_scalar_add(den, psDs[b], EPS)
                rinv = small.tile([1, 1], f32, tag="rinv")
                nc.vector.reciprocal(rinv, den)
                ot = opool.tile([1, R], f32, tag="ot")
                nc.scalar.activation(ot[:, 0:512], psAs[b], AF.Copy, scale=rinv)
                nc.scalar.activation(ot[:, 512:1024], psBs[b], AF.Copy, scale=rinv)
                nc.scalar.dma_start(out[b:b + 1, :], ot)

            # software pipeline: A(b) || B_fin(b-1)+C_mm(b-1) || C_out(b-2)
            tps = [None] * BPC
            for b in range(BPC):
                s_lbas[b] = small.tile([128, NLT], f32, tag=f"slba{b}",
                                       name=f"slba{b}")
                s_embs[b] = small.tile([128, NLT], f32, tag=f"semb{b}",
                                       name=f"semb{b}")
                a_chunk(b, 0)
                c_rnn(b)
                if b >= 1:
                    b_fin(b - 1, *tps[b - 1])
                    c_mm(b - 1)
                if b >= 2:
                    c_out(b - 2)
                for c in range(1, NLT // CHA):
                    a_chunk(b, c)
                tps[b] = b_front(b)
            b_fin(BPC - 1, *tps[BPC - 1])
            c_mm(BPC - 1)
            c_out(BPC - 2)
            c_out(BPC - 1)

    nc.compile()
    return nc


def _get_prog():
    global _PROG
    if _PROG is None:
        _PROG = _build()
    return _PROG


def _qi16(x, q):
    return np.clip(np.round(x * (1.0 / q)), -32767, 32767).astype(np.int16)


def _marshal(embs, prnt_indices, lba_out, rnn_out, W):
    """Host-side input layout: shard over batch, lba->f16 / embs->int16
    into one uint16 container, rnn->fp16, W->int16 transposed, remap idx."""
    lba_q = np.asarray(lba_out, dtype=np.float32).astype(np.float16).view(np.uint16)
    emb_q = _qi16(np.asarray(embs, dtype=np.float32), QX).view(np.uint16)
    xq = np.ascontiguousarray(np.concatenate([lba_q, emb_q], axis=-1))
    rnn = np.asarray(rnn_out, dtype=np.float32).astype(np.float16)
    wTq = np.ascontiguousarray(_qi16(np.asarray(W, dtype=np.float32), QX).T)
    idx = np.asarray(prnt_indices).astype(np.int64)

    pos = ((idx % 128) * NLT + idx // 128).astype(np.uint16)  # [B, L]
    A = pos.reshape(B, 8, 16, 16)
    idxs_w = np.ascontiguousarray(A.transpose(0, 1, 3, 2).reshape(B, 128, NLT))

    in_maps = []
    for c in range(NCORES):
        s = slice(c * BPC, (c + 1) * BPC)
        in_maps.append({
            "xq": xq[s],
            "rnn": rnn[s],
            "wT": wTq,
            "idxs": idxs_w[s],
        })
    return in_maps


def kernel(embs, prnt_indices, lba_out, rnn_out, W):
    global LAST_RESULTS
    from concourse.bass_utils import run_bass_kernel_spmd

    nc = _get_prog()
    in_maps = _marshal(embs, prnt_indices, lba_out, rnn_out, W)
    res = run_bass_kernel_spmd(nc, in_maps, core_ids=list(range(NCORES)))
    LAST_RESULTS = res
    out = np.concatenate([r["out"] for r in res.results], axis=0)
    return out.astype(np.float32)
